# revision 21
# baseline (speedup 1.0000x reference)
"""Llama4TextExperts MoE expert-parallel kernel for 8 Trainium2 NeuronCores.

Per-core (1 expert each): out_e = (up * silu(gate)) @ W2_e where
[gate|up] = h_e @ W1_e.  All matmuls run in bf16 on the PE array with fp32
PSUM accumulation; SwiGLU is fused into the PSUM eviction of the first
matmul chain.

Host-side prep (not HW time): per-expert slices are cast to bf16 and laid
out in the exact SBUF tiling the device DMAs expect:
  - hT:  h_e transposed to [P, KH, T]   (contraction dim H on partitions);
         in the packed (default, "v8") variant the j=0 w1 column block is
         appended to each hT k-row -> [P, KH, T + 2P], so one DMA per
         k-slice feeds both operands of the first matmul chains
  - w1:  gate/up column blocks interleaved -> [KI(-1), P, KH, 2P] so each
         128-row gate block sits next to its up block (fused SwiGLU)
  - w2:  [NH, P, KI, 512] slabs (contraction dim I on partitions)
The device output is the natural [T, H] fp32 layout; the host just
concatenates the 8 per-expert results.

Schedule (see _build_nc_v6): ~7 junk matmuls on a zeroed tile warm the PE
HAM clock gate while the first DMA slices stream; j=0 runs k-major so each
arriving k-slice immediately unlocks 4 matmuls; all data DMAs stay on the
single qSyncDynamicHW FIFO ring in exact consumption order (a second queue
round-robins packets and halves the critical stream's bandwidth).  Steady
state is a dense 216 ns/matmul stream (N=512 bf16 roofline); measured
~685 us/core at 2.4 GHz (the chip sometimes runs the whole kernel at its
2.0 GHz P0 power state, scaling everything by ~1.2x).
"""

import numpy as np

NUM_EXPERTS = 8
HIDDEN = 2048
EXPERT_DIM = 4096
TOK = 1024  # tokens per expert

P = 128
KH = HIDDEN // P        # 16 contraction chunks for matmul 1
KI = EXPERT_DIM // P    # 32 contraction chunks for matmul 2
NT = TOK // 512         # 2  token chunks (psum free dim 512)
NH = HIDDEN // 512      # 4  output-column chunks

TRACE = False           # set by test harness to collect an NTFF profile
LAST_RESULT = None      # BassKernelResults of the most recent run
VARIANT = "v8"          # kernel schedule variant (see _build_nc)

_NC = {}


def _build_nc(variant):
    import concourse.mybir as mybir
    from concourse import bacc, tile
    from concourse.tile_rust import add_dep_helper

    nc = bacc.Bacc("TRN2", target_bir_lowering=False)
    if variant in ("v6", "v7", "v8"):
        return _build_nc_v6(nc, mybir, tile, variant)
    hT_d = nc.dram_tensor("hT", [P, KH, TOK], mybir.dt.bfloat16, kind="ExternalInput")
    w1_d = nc.dram_tensor("w1", [KI, P, KH, 2 * P], mybir.dt.bfloat16, kind="ExternalInput")
    w2_d = nc.dram_tensor("w2", [NH, P, KI, 512], mybir.dt.bfloat16, kind="ExternalInput")
    out_d = nc.dram_tensor("out", [TOK, HIDDEN], mybir.dt.float32, kind="ExternalOutput")

    FT = mybir.dt.float32
    BF = mybir.dt.bfloat16
    KG = 4  # k-chunks per startup DMA (1 MiB hT pieces, 256 KiB w1 pieces)
    with tile.TileContext(nc) as tc:
        with tc.tile_pool(name="ht", bufs=1) as ht_pool, \
             tc.tile_pool(name="w1", bufs=3) as w1_pool, \
             tc.tile_pool(name="gated", bufs=1) as gated_pool, \
             tc.tile_pool(name="w2", bufs=2) as w2_pool, \
             tc.tile_pool(name="tmp", bufs=4) as tmp_pool, \
             tc.tile_pool(name="ob", bufs=4) as ob_pool, \
             tc.tile_pool(name="psum1", bufs=3, space="PSUM") as psum1_pool, \
             tc.tile_pool(name="psum2", bufs=2, space="PSUM") as psum2_pool:

            if variant in ("v4", "v5"):
                # PE warmup: the HAM clock gate runs the PE at 1.2 GHz until
                # it has been busy ~3.4us.  Chew on a zeroed tile while the
                # first input DMAs stream so real matmuls start at 2.4 GHz.
                wz = tmp_pool.tile([P, 512], BF, tag="warm_rhs")
                wl = tmp_pool.tile([P, P], BF, tag="warm_lhs")
                nc.any.memset(wz[:], 0.0)
                nc.any.memset(wl[:], 0.0)
                warm_ps = [psum2_pool.tile([P, 512], FT, tag="po", name=f"warm_{i}")
                           for i in range(2)]
                for i in range(24):
                    nc.tensor.matmul(warm_ps[i % 2][:], wl[:], wz[:],
                                     start=True, stop=True)

            hT = ht_pool.tile([P, KH, TOK], BF)
            gated = gated_pool.tile([P, KI, TOK], BF)

            n_special = 2 if variant == "v4" else 0
            w1t01 = [w1_pool.tile([P, KH, 2 * P], BF, tag="w1t", name=f"w1t_{j}")
                     for j in range(n_special)]

            if variant == "v4":
                # Startup DMAs as coarse chained "waves" in consumption
                # order: concurrent DMAs complete all-together (SDMA
                # round-robins at packet granularity), so unordered the
                # first matmul waits for the LAST startup byte.  Coarse
                # links only — each link costs ~1-2us completion latency.
                w_a = [nc.sync.dma_start(w1t01[0][:], w1_d[0]),
                       nc.sync.dma_start(hT[:, :, 0:512], hT_d[:, :, 0:512])]
                w_b = nc.sync.dma_start(hT[:, :, 512:1024], hT_d[:, :, 512:1024])
                for p in w_a:
                    add_dep_helper(w_b.ins, p.ins, sync=True, reason="wave b")
                w_c = nc.sync.dma_start(w1t01[1][:], w1_d[1])
                add_dep_helper(w_c.ins, w_b.ins, sync=True, reason="wave c")
                prev_wave = [w_c]
            else:
                if n_special:
                    for kg in range(KH // KG):
                        ksl = slice(kg * KG, (kg + 1) * KG)
                        nc.sync.dma_start(w1t01[0][:, ksl, :], w1_d[0, :, ksl, :])
                        nc.sync.dma_start(hT[:, ksl, :], hT_d[:, ksl, :])
                    nc.sync.dma_start(w1t01[1][:], w1_d[1])
                else:
                    for kg in range(KH // KG):
                        ksl = slice(kg * KG, (kg + 1) * KG)
                        nc.sync.dma_start(hT[:, ksl, :], hT_d[:, ksl, :])
                prev_wave = []

            # ---- matmul 1 + fused SwiGLU: gated^T[I, T] ----
            for j in range(KI):
                if j < n_special:
                    # startup: n-outer, gate/up interleaved per k so each
                    # wave's arrival unlocks the next slice of matmuls
                    w1t = w1t01[j]
                    for n in range(NT):
                        tsl = slice(n * 512, (n + 1) * 512)
                        pg = psum1_pool.tile([P, 512], FT, tag="pg", name=f"pg_i{j}_{n}")
                        pu = psum1_pool.tile([P, 512], FT, tag="pu", name=f"pu_i{j}_{n}")
                        for k in range(KH):
                            nc.tensor.matmul(pg[:], w1t[:, k, 0:P], hT[:, k, tsl],
                                             start=(k == 0), stop=(k == KH - 1))
                            nc.tensor.matmul(pu[:], w1t[:, k, P:2 * P], hT[:, k, tsl],
                                             start=(k == 0), stop=(k == KH - 1))
                        sl = tmp_pool.tile([P, 512], BF, tag="silu", name=f"sl_i{j}_{n}")
                        nc.scalar.activation(sl[:], pg[:], mybir.ActivationFunctionType.Silu)
                        nc.vector.tensor_mul(out=gated[:, j, tsl], in0=sl[:], in1=pu[:])
                    continue
                w1t = w1_pool.tile([P, KH, 2 * P], BF, tag="w1t")
                di = nc.sync.dma_start(w1t[:], w1_d[j])
                if j == n_special and prev_wave:
                    # keep this slab load out of the startup waves' bandwidth
                    for p in prev_wave:
                        add_dep_helper(di.ins, p.ins, sync=True, reason="after startup waves")
                for n in range(NT):
                    tsl = slice(n * 512, (n + 1) * 512)
                    pg = psum1_pool.tile([P, 512], FT, tag="pg")
                    pu = psum1_pool.tile([P, 512], FT, tag="pu")
                    for k in range(KH):
                        nc.tensor.matmul(pg[:], w1t[:, k, 0:P], hT[:, k, tsl],
                                         start=(k == 0), stop=(k == KH - 1))
                    for k in range(KH):
                        nc.tensor.matmul(pu[:], w1t[:, k, P:2 * P], hT[:, k, tsl],
                                         start=(k == 0), stop=(k == KH - 1))
                    sl = tmp_pool.tile([P, 512], BF, tag="silu")
                    nc.scalar.activation(sl[:], pg[:], mybir.ActivationFunctionType.Silu)
                    nc.vector.tensor_mul(out=gated[:, j, tsl], in0=sl[:], in1=pu[:])

            # ---- matmul 2: out[T, H] = gated @ W2 ----
            for hc in range(NH):
                w2t = w2_pool.tile([P, KI, 512], BF)
                nc.sync.dma_start(w2t[:], w2_d[hc])
                for t in range(TOK // P):
                    po = psum2_pool.tile([P, 512], FT, tag="po")
                    for i in range(KI):
                        nc.tensor.matmul(po[:], gated[:, i, t * P:(t + 1) * P],
                                         w2t[:, i, :],
                                         start=(i == 0), stop=(i == KI - 1))
                    ob = ob_pool.tile([P, 512], FT, tag="ob")
                    nc.vector.tensor_copy(ob[:], po[:])
                    nc.sync.dma_start(out_d[t * P:(t + 1) * P, hc * 512:(hc + 1) * 512], ob[:])

    nc.compile()
    return nc


def _build_nc_v6(nc, mybir, tile, variant="v6"):
    """v6 schedule.

    Startup: all data DMAs funnel through the single qSyncDynamicHW FIFO
    ring, so issue order == transfer order — no dependency-chained waves
    (each sync link in v4 cost ~1-2us of queue idle).  hT and the j=0 w1
    slab stream as interleaved 2-k-chunk slices in consumption order, and
    j=0's matmul chains run k-major so each arriving slice immediately
    unlocks 4 matmuls.  Real (cold) matmuls double as the HAM warmup, so
    the zero-tile warmup spin of v4 is gone.

    Steady state: all of mm1 is k-major per (j): for each k, the gate
    column block is loaded once and used for both token chunks, then the
    up block — halving LDWEIGHTS traffic.  Each j holds 4 PSUM banks
    (gate/up x 2 token chunks); evictions overlap the next j's chains.

    Tail: the final (hc, t) eviction is split into two 256-column halves
    so the last output DMA is half-sized and starts one copy earlier.
    """
    from concourse.tile_rust import add_dep_helper  # noqa: F401 (kept for parity)

    packed = variant == "v8"
    if packed:
        # hT rows carry the j=0 w1 column block inline (host-packed), so
        # one DMA per k-slice feeds both operands of the j=0 chains.
        hT_d = nc.dram_tensor("hT", [P, KH, TOK + 2 * P], mybir.dt.bfloat16, kind="ExternalInput")
        w1_d = nc.dram_tensor("w1", [KI - 1, P, KH, 2 * P], mybir.dt.bfloat16, kind="ExternalInput")
    else:
        hT_d = nc.dram_tensor("hT", [P, KH, TOK], mybir.dt.bfloat16, kind="ExternalInput")
        w1_d = nc.dram_tensor("w1", [KI, P, KH, 2 * P], mybir.dt.bfloat16, kind="ExternalInput")
    w2_d = nc.dram_tensor("w2", [NH, P, KI, 512], mybir.dt.bfloat16, kind="ExternalInput")
    out_d = nc.dram_tensor("out", [TOK, HIDDEN], mybir.dt.float32, kind="ExternalOutput")

    FT = mybir.dt.float32
    BF = mybir.dt.bfloat16
    KG = 2  # k-chunks per startup DMA slice (512 KiB hT / 128 KiB w1 pieces)
    with tile.TileContext(nc) as tc:
        with tc.tile_pool(name="ht", bufs=1) as ht_pool, \
             tc.tile_pool(name="w1", bufs=3) as w1_pool, \
             tc.tile_pool(name="gated", bufs=1) as gated_pool, \
             tc.tile_pool(name="w2", bufs=2) as w2_pool, \
             tc.tile_pool(name="tmp", bufs=4) as tmp_pool, \
             tc.tile_pool(name="ob", bufs=4) as ob_pool, \
             tc.tile_pool(name="psum1", bufs=3, space="PSUM") as psum1_pool, \
             tc.tile_pool(name="psum2", bufs=2, space="PSUM") as psum2_pool:

            hT = ht_pool.tile([P, KH, TOK + 2 * P] if packed else [P, KH, TOK], BF)
            gated = gated_pool.tile([P, KI, TOK], BF)

            n_special = 1 if packed else 2
            w1t01 = [w1_pool.tile([P, KH, 2 * P], BF, tag="w1t", name=f"w1t_{j}")
                     for j in range(n_special)]

            if variant in ("v7", "v8"):
                # HAM pre-warm: junk matmuls on a zeroed tile keep the
                # PE busy through the startup DMA wait, so real matmuls
                # start at 2.4 GHz instead of paying the 1.2 GHz ramp.
                # (The zero tile is memset on GpSimd — scalar is blocked
                # by ACT table loads early on.)
                wz = tmp_pool.tile([P, 512], BF, tag="warm_rhs", bufs=1)
                nc.gpsimd.memset(wz[:], 0.0)
                warm_ps = [psum2_pool.tile([P, 512], FT, tag="po", name=f"warm_{i}")
                           for i in range(2)]
                for i in range(7):
                    nc.tensor.matmul(warm_ps[i % 2][:], wz[:, 0:P], wz[:],
                                     start=True, stop=True)
            # Startup DMAs in exact consumption order on the sync queue.
            # (All on ONE queue: a second SWDGE queue round-robins packets
            # and halves the critical stream's bandwidth — measured.)
            if packed:
                # One DMA per k-slice carries tokens + the j=0 weights:
                # delivery (~0.75us/k) stays ahead of consumption
                # (~0.86us/k), so j=0 never stalls after its first slice.
                # The j=1 slab goes out after k11 — early enough to beat
                # j=1's start, late enough that k12-15 still arrive in
                # time for the tail of j=0's chains.
                for k in range(12):
                    nc.sync.dma_start(hT[:, k:k + 1, :], hT_d[:, k:k + 1, :])
                nc.sync.dma_start(w1t01[0][:], w1_d[0])
                for k in range(12, KH):
                    nc.sync.dma_start(hT[:, k:k + 1, :], hT_d[:, k:k + 1, :])
            else:
                if variant == "v7":
                    ksls = [slice(0, 1), slice(1, 3), slice(3, 5), slice(5, 7),
                            slice(7, 9), slice(9, 11), slice(11, 13), slice(13, 16)]
                else:
                    ksls = [slice(c * KG, (c + 1) * KG) for c in range(KH // KG)]
                for ksl in ksls:
                    nc.sync.dma_start(hT[:, ksl, :], hT_d[:, ksl, :])
                    nc.sync.dma_start(w1t01[0][:, ksl, :], w1_d[0, :, ksl, :])
                nc.sync.dma_start(w1t01[1][:], w1_d[1])

            # ---- matmul 1 + fused SwiGLU: gated^T[I, T], all j k-major ----
            for j in range(KI):
                if packed:
                    if j == 0:
                        w1t = None  # weights live in the packed hT rows
                    elif j == 1:
                        w1t = w1t01[0]
                    else:
                        w1t = w1_pool.tile([P, KH, 2 * P], BF, tag="w1t")
                        nc.sync.dma_start(w1t[:], w1_d[j - 1])
                else:
                    if j < 2:
                        w1t = w1t01[j]
                    else:
                        w1t = w1_pool.tile([P, KH, 2 * P], BF, tag="w1t")
                        nc.sync.dma_start(w1t[:], w1_d[j])
                pg = [psum1_pool.tile([P, 512], FT, tag="pg", name=f"pg_j{j}_{n}")
                      for n in range(NT)]
                pu = [psum1_pool.tile([P, 512], FT, tag="pu", name=f"pu_j{j}_{n}")
                      for n in range(NT)]
                for k in range(KH):
                    st, sp = (k == 0), (k == KH - 1)
                    if w1t is None:
                        wg = hT[:, k, TOK:TOK + P]
                        wu = hT[:, k, TOK + P:TOK + 2 * P]
                    else:
                        wg = w1t[:, k, 0:P]
                        wu = w1t[:, k, P:2 * P]
                    for n in range(NT):
                        nc.tensor.matmul(pg[n][:], wg,
                                         hT[:, k, n * 512:(n + 1) * 512],
                                         start=st, stop=sp)
                    for n in range(NT):
                        nc.tensor.matmul(pu[n][:], wu,
                                         hT[:, k, n * 512:(n + 1) * 512],
                                         start=st, stop=sp)
                for n in range(NT):
                    tsl = slice(n * 512, (n + 1) * 512)
                    sl = tmp_pool.tile([P, 512], BF, tag="silu", name=f"sl_j{j}_{n}")
                    nc.scalar.activation(sl[:], pg[n][:], mybir.ActivationFunctionType.Silu)
                    nc.vector.tensor_mul(out=gated[:, j, tsl], in0=sl[:], in1=pu[n][:])

            # ---- matmul 2: out[T, H] = gated @ W2 ----
            for hc in range(NH):
                w2t = w2_pool.tile([P, KI, 512], BF)
                nc.sync.dma_start(w2t[:], w2_d[hc])
                for t in range(TOK // P):
                    po = psum2_pool.tile([P, 512], FT, tag="po")
                    for i in range(KI):
                        nc.tensor.matmul(po[:], gated[:, i, t * P:(t + 1) * P],
                                         w2t[:, i, :],
                                         start=(i == 0), stop=(i == KI - 1))
                    rsl = slice(t * P, (t + 1) * P)
                    if hc == NH - 1 and t == TOK // P - 1:
                        # tail: two half-width evictions so the last DMA
                        # is smaller and starts a copy earlier
                        ob = ob_pool.tile([P, 512], FT, tag="ob")
                        nc.vector.tensor_copy(ob[:, 0:256], po[:, 0:256])
                        nc.sync.dma_start(out_d[rsl, hc * 512:hc * 512 + 256], ob[:, 0:256])
                        nc.vector.tensor_copy(ob[:, 256:512], po[:, 256:512])
                        nc.sync.dma_start(out_d[rsl, hc * 512 + 256:(hc + 1) * 512], ob[:, 256:512])
                    else:
                        ob = ob_pool.tile([P, 512], FT, tag="ob")
                        nc.vector.tensor_copy(ob[:], po[:])
                        nc.sync.dma_start(out_d[rsl, hc * 512:(hc + 1) * 512], ob[:])

    nc.compile()
    return nc


def _get_nc():
    if VARIANT not in _NC:
        _NC[VARIANT] = _build_nc(VARIANT)
    return _NC[VARIANT]


def kernel(hidden_states, gate_up_proj, down_proj):
    import ml_dtypes
    from concourse.bass_utils import run_bass_kernel_spmd

    global LAST_RESULT
    bf16 = ml_dtypes.bfloat16

    h = np.asarray(hidden_states, dtype=np.float32)
    w1 = np.asarray(gate_up_proj, dtype=np.float32)
    w2 = np.asarray(down_proj, dtype=np.float32)
    assert h.shape == (NUM_EXPERTS * TOK, HIDDEN)
    assert w1.shape == (NUM_EXPERTS, HIDDEN, 2 * EXPERT_DIM)
    assert w2.shape == (NUM_EXPERTS, EXPERT_DIM, HIDDEN)

    nc = _get_nc()

    packed = VARIANT == "v8"
    in_maps = []
    for e in range(NUM_EXPERTS):
        he = h[e * TOK:(e + 1) * TOK]                       # [T, H]
        # [H, T] -> [KH, P, T] -> [P, KH, T]
        hT_e = he.T.reshape(KH, P, TOK).transpose(1, 0, 2).astype(bf16)
        # [H, 2I]: col = gu*I + j*P + m -> [j, p, ko, gu*P + m]
        w1_e = (w1[e].reshape(KH, P, 2, KI, P)
                .transpose(3, 1, 0, 2, 4)
                .reshape(KI, P, KH, 2 * P)
                .astype(bf16))
        # [I, H]: row = ki*P + p, col = hc*512 + c -> [hc, p, ki, c]
        w2_e = (w2[e].reshape(KI, P, NH, 512)
                .transpose(2, 1, 0, 3)
                .reshape(NH, P, KI, 512)
                .astype(bf16))
        if packed:
            # pack the j=0 w1 column block into the hT rows so one DMA
            # per k-slice feeds both operands of the j=0 chains
            hT_e = np.concatenate([hT_e, w1_e[0]], axis=-1)  # [P, KH, T+2P]
            w1_e = np.ascontiguousarray(w1_e[1:])            # [KI-1, P, KH, 2P]
        in_maps.append({"hT": hT_e, "w1": w1_e, "w2": w2_e})

    res = run_bass_kernel_spmd(nc, in_maps, list(range(NUM_EXPERTS)), trace=TRACE)
    LAST_RESULT = res

    out = np.concatenate([res.results[e]["out"] for e in range(NUM_EXPERTS)], axis=0)
    return out.astype(np.float32)



# revision 22
# speedup vs baseline: 1.0004x; 1.0004x over previous
"""Llama4TextExperts MoE expert-parallel kernel for 8 Trainium2 NeuronCores.

Per-core (1 expert each): out_e = (up * silu(gate)) @ W2_e where
[gate|up] = h_e @ W1_e.  All matmuls run in bf16 on the PE array with fp32
PSUM accumulation; SwiGLU is fused into the PSUM eviction of the first
matmul chain.

Host-side prep (not HW time): per-expert slices are cast to bf16 and laid
out in the exact SBUF tiling the device DMAs expect:
  - hT:  h_e transposed to [P, KH, T]   (contraction dim H on partitions);
         in the packed (default, "v8") variant the j=0 w1 column block is
         appended to each hT k-row -> [P, KH, T + 2P], so one DMA per
         k-slice feeds both operands of the first matmul chains
  - w1:  gate/up column blocks interleaved -> [KI(-1), P, KH, 2P] so each
         128-row gate block sits next to its up block (fused SwiGLU)
  - w2:  [NH, P, KI, 512] slabs (contraction dim I on partitions)
The device output is the natural [T, H] fp32 layout; the host just
concatenates the 8 per-expert results.

Schedule (see _build_nc_v6): ~7 junk matmuls on a zeroed tile warm the PE
HAM clock gate while the first DMA slices stream; j=0 runs k-major so each
arriving k-slice immediately unlocks 4 matmuls; all data DMAs stay on the
single qSyncDynamicHW FIFO ring in exact consumption order (a second queue
round-robins packets and halves the critical stream's bandwidth).  Steady
state is a dense 216 ns/matmul stream (N=512 bf16 roofline); measured
~685 us/core at 2.4 GHz (the chip sometimes runs the whole kernel at its
2.0 GHz P0 power state, scaling everything by ~1.2x).
"""

import numpy as np

NUM_EXPERTS = 8
HIDDEN = 2048
EXPERT_DIM = 4096
TOK = 1024  # tokens per expert

P = 128
KH = HIDDEN // P        # 16 contraction chunks for matmul 1
KI = EXPERT_DIM // P    # 32 contraction chunks for matmul 2
NT = TOK // 512         # 2  token chunks (psum free dim 512)
NH = HIDDEN // 512      # 4  output-column chunks

TRACE = False           # set by test harness to collect an NTFF profile
LAST_RESULT = None      # BassKernelResults of the most recent run
VARIANT = "v8"          # kernel schedule variant (see _build_nc)

_NC = {}


def _build_nc(variant):
    import concourse.mybir as mybir
    from concourse import bacc, tile
    from concourse.tile_rust import add_dep_helper

    nc = bacc.Bacc("TRN2", target_bir_lowering=False)
    if variant in ("v6", "v7", "v8"):
        return _build_nc_v6(nc, mybir, tile, variant)
    hT_d = nc.dram_tensor("hT", [P, KH, TOK], mybir.dt.bfloat16, kind="ExternalInput")
    w1_d = nc.dram_tensor("w1", [KI, P, KH, 2 * P], mybir.dt.bfloat16, kind="ExternalInput")
    w2_d = nc.dram_tensor("w2", [NH, P, KI, 512], mybir.dt.bfloat16, kind="ExternalInput")
    out_d = nc.dram_tensor("out", [TOK, HIDDEN], mybir.dt.float32, kind="ExternalOutput")

    FT = mybir.dt.float32
    BF = mybir.dt.bfloat16
    KG = 4  # k-chunks per startup DMA (1 MiB hT pieces, 256 KiB w1 pieces)
    with tile.TileContext(nc) as tc:
        with tc.tile_pool(name="ht", bufs=1) as ht_pool, \
             tc.tile_pool(name="w1", bufs=3) as w1_pool, \
             tc.tile_pool(name="gated", bufs=1) as gated_pool, \
             tc.tile_pool(name="w2", bufs=2) as w2_pool, \
             tc.tile_pool(name="tmp", bufs=4) as tmp_pool, \
             tc.tile_pool(name="ob", bufs=4) as ob_pool, \
             tc.tile_pool(name="psum1", bufs=3, space="PSUM") as psum1_pool, \
             tc.tile_pool(name="psum2", bufs=2, space="PSUM") as psum2_pool:

            if variant in ("v4", "v5"):
                # PE warmup: the HAM clock gate runs the PE at 1.2 GHz until
                # it has been busy ~3.4us.  Chew on a zeroed tile while the
                # first input DMAs stream so real matmuls start at 2.4 GHz.
                wz = tmp_pool.tile([P, 512], BF, tag="warm_rhs")
                wl = tmp_pool.tile([P, P], BF, tag="warm_lhs")
                nc.any.memset(wz[:], 0.0)
                nc.any.memset(wl[:], 0.0)
                warm_ps = [psum2_pool.tile([P, 512], FT, tag="po", name=f"warm_{i}")
                           for i in range(2)]
                for i in range(24):
                    nc.tensor.matmul(warm_ps[i % 2][:], wl[:], wz[:],
                                     start=True, stop=True)

            hT = ht_pool.tile([P, KH, TOK], BF)
            gated = gated_pool.tile([P, KI, TOK], BF)

            n_special = 2 if variant == "v4" else 0
            w1t01 = [w1_pool.tile([P, KH, 2 * P], BF, tag="w1t", name=f"w1t_{j}")
                     for j in range(n_special)]

            if variant == "v4":
                # Startup DMAs as coarse chained "waves" in consumption
                # order: concurrent DMAs complete all-together (SDMA
                # round-robins at packet granularity), so unordered the
                # first matmul waits for the LAST startup byte.  Coarse
                # links only — each link costs ~1-2us completion latency.
                w_a = [nc.sync.dma_start(w1t01[0][:], w1_d[0]),
                       nc.sync.dma_start(hT[:, :, 0:512], hT_d[:, :, 0:512])]
                w_b = nc.sync.dma_start(hT[:, :, 512:1024], hT_d[:, :, 512:1024])
                for p in w_a:
                    add_dep_helper(w_b.ins, p.ins, sync=True, reason="wave b")
                w_c = nc.sync.dma_start(w1t01[1][:], w1_d[1])
                add_dep_helper(w_c.ins, w_b.ins, sync=True, reason="wave c")
                prev_wave = [w_c]
            else:
                if n_special:
                    for kg in range(KH // KG):
                        ksl = slice(kg * KG, (kg + 1) * KG)
                        nc.sync.dma_start(w1t01[0][:, ksl, :], w1_d[0, :, ksl, :])
                        nc.sync.dma_start(hT[:, ksl, :], hT_d[:, ksl, :])
                    nc.sync.dma_start(w1t01[1][:], w1_d[1])
                else:
                    for kg in range(KH // KG):
                        ksl = slice(kg * KG, (kg + 1) * KG)
                        nc.sync.dma_start(hT[:, ksl, :], hT_d[:, ksl, :])
                prev_wave = []

            # ---- matmul 1 + fused SwiGLU: gated^T[I, T] ----
            for j in range(KI):
                if j < n_special:
                    # startup: n-outer, gate/up interleaved per k so each
                    # wave's arrival unlocks the next slice of matmuls
                    w1t = w1t01[j]
                    for n in range(NT):
                        tsl = slice(n * 512, (n + 1) * 512)
                        pg = psum1_pool.tile([P, 512], FT, tag="pg", name=f"pg_i{j}_{n}")
                        pu = psum1_pool.tile([P, 512], FT, tag="pu", name=f"pu_i{j}_{n}")
                        for k in range(KH):
                            nc.tensor.matmul(pg[:], w1t[:, k, 0:P], hT[:, k, tsl],
                                             start=(k == 0), stop=(k == KH - 1))
                            nc.tensor.matmul(pu[:], w1t[:, k, P:2 * P], hT[:, k, tsl],
                                             start=(k == 0), stop=(k == KH - 1))
                        sl = tmp_pool.tile([P, 512], BF, tag="silu", name=f"sl_i{j}_{n}")
                        nc.scalar.activation(sl[:], pg[:], mybir.ActivationFunctionType.Silu)
                        nc.vector.tensor_mul(out=gated[:, j, tsl], in0=sl[:], in1=pu[:])
                    continue
                w1t = w1_pool.tile([P, KH, 2 * P], BF, tag="w1t")
                di = nc.sync.dma_start(w1t[:], w1_d[j])
                if j == n_special and prev_wave:
                    # keep this slab load out of the startup waves' bandwidth
                    for p in prev_wave:
                        add_dep_helper(di.ins, p.ins, sync=True, reason="after startup waves")
                for n in range(NT):
                    tsl = slice(n * 512, (n + 1) * 512)
                    pg = psum1_pool.tile([P, 512], FT, tag="pg")
                    pu = psum1_pool.tile([P, 512], FT, tag="pu")
                    for k in range(KH):
                        nc.tensor.matmul(pg[:], w1t[:, k, 0:P], hT[:, k, tsl],
                                         start=(k == 0), stop=(k == KH - 1))
                    for k in range(KH):
                        nc.tensor.matmul(pu[:], w1t[:, k, P:2 * P], hT[:, k, tsl],
                                         start=(k == 0), stop=(k == KH - 1))
                    sl = tmp_pool.tile([P, 512], BF, tag="silu")
                    nc.scalar.activation(sl[:], pg[:], mybir.ActivationFunctionType.Silu)
                    nc.vector.tensor_mul(out=gated[:, j, tsl], in0=sl[:], in1=pu[:])

            # ---- matmul 2: out[T, H] = gated @ W2 ----
            for hc in range(NH):
                w2t = w2_pool.tile([P, KI, 512], BF)
                nc.sync.dma_start(w2t[:], w2_d[hc])
                for t in range(TOK // P):
                    po = psum2_pool.tile([P, 512], FT, tag="po")
                    for i in range(KI):
                        nc.tensor.matmul(po[:], gated[:, i, t * P:(t + 1) * P],
                                         w2t[:, i, :],
                                         start=(i == 0), stop=(i == KI - 1))
                    ob = ob_pool.tile([P, 512], FT, tag="ob")
                    nc.vector.tensor_copy(ob[:], po[:])
                    nc.sync.dma_start(out_d[t * P:(t + 1) * P, hc * 512:(hc + 1) * 512], ob[:])

    nc.compile()
    return nc


def _build_nc_v6(nc, mybir, tile, variant="v6"):
    """v6 schedule.

    Startup: all data DMAs funnel through the single qSyncDynamicHW FIFO
    ring, so issue order == transfer order — no dependency-chained waves
    (each sync link in v4 cost ~1-2us of queue idle).  hT and the j=0 w1
    slab stream as interleaved 2-k-chunk slices in consumption order, and
    j=0's matmul chains run k-major so each arriving slice immediately
    unlocks 4 matmuls.  Real (cold) matmuls double as the HAM warmup, so
    the zero-tile warmup spin of v4 is gone.

    Steady state: all of mm1 is k-major per (j): for each k, the gate
    column block is loaded once and used for both token chunks, then the
    up block — halving LDWEIGHTS traffic.  Each j holds 4 PSUM banks
    (gate/up x 2 token chunks); evictions overlap the next j's chains.

    Tail: the final (hc, t) eviction is split into two 256-column halves
    so the last output DMA is half-sized and starts one copy earlier.
    """
    from concourse.tile_rust import add_dep_helper  # noqa: F401 (kept for parity)

    packed = variant == "v8"
    if packed:
        # hT rows carry the j=0 w1 column block inline (host-packed), so
        # one DMA per k-slice feeds both operands of the j=0 chains.
        hT_d = nc.dram_tensor("hT", [P, KH, TOK + 2 * P], mybir.dt.bfloat16, kind="ExternalInput")
        w1_d = nc.dram_tensor("w1", [KI - 1, P, KH, 2 * P], mybir.dt.bfloat16, kind="ExternalInput")
    else:
        hT_d = nc.dram_tensor("hT", [P, KH, TOK], mybir.dt.bfloat16, kind="ExternalInput")
        w1_d = nc.dram_tensor("w1", [KI, P, KH, 2 * P], mybir.dt.bfloat16, kind="ExternalInput")
    w2_d = nc.dram_tensor("w2", [NH, P, KI, 512], mybir.dt.bfloat16, kind="ExternalInput")
    out_d = nc.dram_tensor("out", [TOK, HIDDEN], mybir.dt.float32, kind="ExternalOutput")

    FT = mybir.dt.float32
    BF = mybir.dt.bfloat16
    KG = 2  # k-chunks per startup DMA slice (512 KiB hT / 128 KiB w1 pieces)
    with tile.TileContext(nc) as tc:
        with tc.tile_pool(name="ht", bufs=1) as ht_pool, \
             tc.tile_pool(name="w1", bufs=3) as w1_pool, \
             tc.tile_pool(name="gated", bufs=1) as gated_pool, \
             tc.tile_pool(name="w2", bufs=2) as w2_pool, \
             tc.tile_pool(name="tmp", bufs=4) as tmp_pool, \
             tc.tile_pool(name="ob", bufs=4) as ob_pool, \
             tc.tile_pool(name="psum1", bufs=3, space="PSUM") as psum1_pool, \
             tc.tile_pool(name="psum2", bufs=2, space="PSUM") as psum2_pool:

            hT = ht_pool.tile([P, KH, TOK + 2 * P] if packed else [P, KH, TOK], BF)
            gated = gated_pool.tile([P, KI, TOK], BF)

            n_special = 1 if packed else 2
            w1t01 = [w1_pool.tile([P, KH, 2 * P], BF, tag="w1t", name=f"w1t_{j}")
                     for j in range(n_special)]

            if variant in ("v7", "v8"):
                # HAM pre-warm: junk matmuls on a zeroed tile keep the
                # PE busy through the startup DMA wait, so real matmuls
                # start at 2.4 GHz instead of paying the 1.2 GHz ramp.
                # (The zero tile is memset on GpSimd — scalar is blocked
                # by ACT table loads early on.)
                wz = tmp_pool.tile([P, 512], BF, tag="warm_rhs", bufs=1)
                nc.gpsimd.memset(wz[:], 0.0)
                warm_ps = [psum2_pool.tile([P, 512], FT, tag="po", name=f"warm_{i}")
                           for i in range(2)]
                for i in range(7):
                    nc.tensor.matmul(warm_ps[i % 2][:], wz[:, 0:P], wz[:],
                                     start=True, stop=True)
            # Startup DMAs in exact consumption order on the sync queue.
            # (All on ONE queue: a second SWDGE queue round-robins packets
            # and halves the critical stream's bandwidth — measured.)
            if packed:
                # One DMA per k-slice carries tokens + the j=0 weights:
                # delivery (~0.75us/k) stays ahead of consumption
                # (~0.86us/k), so j=0 never stalls after its first slice.
                # The j=1 slab streams as two halves interleaved with the
                # last hT slices — small enough not to delay k12-15 past
                # their j=0 deadlines, early enough to beat j=1's start.
                for k in range(12):
                    nc.sync.dma_start(hT[:, k:k + 1, :], hT_d[:, k:k + 1, :])
                nc.sync.dma_start(w1t01[0][:, 0:KH // 2, :], w1_d[0, :, 0:KH // 2, :])
                nc.sync.dma_start(hT[:, 12:13, :], hT_d[:, 12:13, :])
                nc.sync.dma_start(hT[:, 13:14, :], hT_d[:, 13:14, :])
                nc.sync.dma_start(w1t01[0][:, KH // 2:KH, :], w1_d[0, :, KH // 2:KH, :])
                nc.sync.dma_start(hT[:, 14:15, :], hT_d[:, 14:15, :])
                nc.sync.dma_start(hT[:, 15:16, :], hT_d[:, 15:16, :])
            else:
                if variant == "v7":
                    ksls = [slice(0, 1), slice(1, 3), slice(3, 5), slice(5, 7),
                            slice(7, 9), slice(9, 11), slice(11, 13), slice(13, 16)]
                else:
                    ksls = [slice(c * KG, (c + 1) * KG) for c in range(KH // KG)]
                for ksl in ksls:
                    nc.sync.dma_start(hT[:, ksl, :], hT_d[:, ksl, :])
                    nc.sync.dma_start(w1t01[0][:, ksl, :], w1_d[0, :, ksl, :])
                nc.sync.dma_start(w1t01[1][:], w1_d[1])

            # ---- matmul 1 + fused SwiGLU: gated^T[I, T], all j k-major ----
            for j in range(KI):
                if packed:
                    if j == 0:
                        w1t = None  # weights live in the packed hT rows
                    elif j == 1:
                        w1t = w1t01[0]
                    else:
                        w1t = w1_pool.tile([P, KH, 2 * P], BF, tag="w1t")
                        nc.sync.dma_start(w1t[:], w1_d[j - 1])
                else:
                    if j < 2:
                        w1t = w1t01[j]
                    else:
                        w1t = w1_pool.tile([P, KH, 2 * P], BF, tag="w1t")
                        nc.sync.dma_start(w1t[:], w1_d[j])
                pg = [psum1_pool.tile([P, 512], FT, tag="pg", name=f"pg_j{j}_{n}")
                      for n in range(NT)]
                pu = [psum1_pool.tile([P, 512], FT, tag="pu", name=f"pu_j{j}_{n}")
                      for n in range(NT)]
                for k in range(KH):
                    st, sp = (k == 0), (k == KH - 1)
                    if w1t is None:
                        wg = hT[:, k, TOK:TOK + P]
                        wu = hT[:, k, TOK + P:TOK + 2 * P]
                    else:
                        wg = w1t[:, k, 0:P]
                        wu = w1t[:, k, P:2 * P]
                    for n in range(NT):
                        nc.tensor.matmul(pg[n][:], wg,
                                         hT[:, k, n * 512:(n + 1) * 512],
                                         start=st, stop=sp)
                    for n in range(NT):
                        nc.tensor.matmul(pu[n][:], wu,
                                         hT[:, k, n * 512:(n + 1) * 512],
                                         start=st, stop=sp)
                for n in range(NT):
                    tsl = slice(n * 512, (n + 1) * 512)
                    sl = tmp_pool.tile([P, 512], BF, tag="silu", name=f"sl_j{j}_{n}")
                    nc.scalar.activation(sl[:], pg[n][:], mybir.ActivationFunctionType.Silu)
                    nc.vector.tensor_mul(out=gated[:, j, tsl], in0=sl[:], in1=pu[n][:])

            # ---- matmul 2: out[T, H] = gated @ W2 ----
            for hc in range(NH):
                w2t = w2_pool.tile([P, KI, 512], BF)
                nc.sync.dma_start(w2t[:], w2_d[hc])
                for t in range(TOK // P):
                    po = psum2_pool.tile([P, 512], FT, tag="po")
                    for i in range(KI):
                        nc.tensor.matmul(po[:], gated[:, i, t * P:(t + 1) * P],
                                         w2t[:, i, :],
                                         start=(i == 0), stop=(i == KI - 1))
                    rsl = slice(t * P, (t + 1) * P)
                    if hc == NH - 1 and t == TOK // P - 1:
                        # tail: two half-width evictions so the last DMA
                        # is smaller and starts a copy earlier
                        ob = ob_pool.tile([P, 512], FT, tag="ob")
                        nc.vector.tensor_copy(ob[:, 0:256], po[:, 0:256])
                        nc.sync.dma_start(out_d[rsl, hc * 512:hc * 512 + 256], ob[:, 0:256])
                        nc.vector.tensor_copy(ob[:, 256:512], po[:, 256:512])
                        nc.sync.dma_start(out_d[rsl, hc * 512 + 256:(hc + 1) * 512], ob[:, 256:512])
                    else:
                        ob = ob_pool.tile([P, 512], FT, tag="ob")
                        nc.vector.tensor_copy(ob[:], po[:])
                        nc.sync.dma_start(out_d[rsl, hc * 512:(hc + 1) * 512], ob[:])

    nc.compile()
    return nc


def _get_nc():
    if VARIANT not in _NC:
        _NC[VARIANT] = _build_nc(VARIANT)
    return _NC[VARIANT]


def kernel(hidden_states, gate_up_proj, down_proj):
    import ml_dtypes
    from concourse.bass_utils import run_bass_kernel_spmd

    global LAST_RESULT
    bf16 = ml_dtypes.bfloat16

    h = np.asarray(hidden_states, dtype=np.float32)
    w1 = np.asarray(gate_up_proj, dtype=np.float32)
    w2 = np.asarray(down_proj, dtype=np.float32)
    assert h.shape == (NUM_EXPERTS * TOK, HIDDEN)
    assert w1.shape == (NUM_EXPERTS, HIDDEN, 2 * EXPERT_DIM)
    assert w2.shape == (NUM_EXPERTS, EXPERT_DIM, HIDDEN)

    nc = _get_nc()

    packed = VARIANT == "v8"
    in_maps = []
    for e in range(NUM_EXPERTS):
        he = h[e * TOK:(e + 1) * TOK]                       # [T, H]
        # [H, T] -> [KH, P, T] -> [P, KH, T]
        hT_e = he.T.reshape(KH, P, TOK).transpose(1, 0, 2).astype(bf16)
        # [H, 2I]: col = gu*I + j*P + m -> [j, p, ko, gu*P + m]
        w1_e = (w1[e].reshape(KH, P, 2, KI, P)
                .transpose(3, 1, 0, 2, 4)
                .reshape(KI, P, KH, 2 * P)
                .astype(bf16))
        # [I, H]: row = ki*P + p, col = hc*512 + c -> [hc, p, ki, c]
        w2_e = (w2[e].reshape(KI, P, NH, 512)
                .transpose(2, 1, 0, 3)
                .reshape(NH, P, KI, 512)
                .astype(bf16))
        if packed:
            # pack the j=0 w1 column block into the hT rows so one DMA
            # per k-slice feeds both operands of the j=0 chains
            hT_e = np.concatenate([hT_e, w1_e[0]], axis=-1)  # [P, KH, T+2P]
            w1_e = np.ascontiguousarray(w1_e[1:])            # [KI-1, P, KH, 2P]
        in_maps.append({"hT": hT_e, "w1": w1_e, "w2": w2_e})

    res = run_bass_kernel_spmd(nc, in_maps, list(range(NUM_EXPERTS)), trace=TRACE)
    LAST_RESULT = res

    out = np.concatenate([res.results[e]["out"] for e in range(NUM_EXPERTS)], axis=0)
    return out.astype(np.float32)

